# revision 2
# baseline (speedup 1.0000x reference)
"""GNN message passing via dense aggregation-matrix matmul (8 cores, SPMD).

Reference: h_t = relu(mean_k h_{t-1}[adj[k,n]] @ W[t] + b[t]), out = [h0,h1,h2].

The neighbor gather+mean is multiplication by a sparse count matrix C
([N_out, N_in], 32 nonzeros per row, values = duplicate counts):
    agg = (C @ H) / 32;  h = relu(agg @ (W/32') ...) with 1/32 folded into W.
SWDGE per-row gathers run at ~1.4us/row on this HW (236ms total), so instead
we materialize C^T densely per core (host-side, from the static adjacency)
and stream it through the PE at full DMA bandwidth:

Per core (2500 output nodes, padded to 2560 = 20 blocks of 128):
    psum[d, n] += H_block[p=j, d].T @ CT_block[p=j, n]   (160 j-blocks, fp8 CT)
    h[n, e] = relu(aggT[d, n].T @ (W/32)[d, e] + b)      (20 n-tiles)
Node coords use the AllGather-padded global space j' = core*2560 + local
for BOTH layers, so one CT layout serves layer 1 (src = padded graph) and
layer 2 (src = AllGathered h1).
"""

import numpy as np
import ml_dtypes

import concourse.bacc as bacc
import concourse.mybir as mybir
import concourse.tile as tile
from concourse.bass_utils import run_bass_kernel_spmd

N, K, D = 20000, 32, 128
NCORES = 8
NS = N // NCORES  # 2500 real nodes per core
CHUNK = 128
NCH = (NS + CHUNK - 1) // CHUNK  # 20 blocks per core
NSP = NCH * CHUNK  # 2560 padded nodes per core
NB = NCORES * NCH  # 160 global padded j-blocks
GRP = 8  # A^T blocks per DMA
NGRP = NB // GRP  # 40 DMA groups per layer

A_DT = mybir.dt.float8e4
A_NP_DT = ml_dtypes.float8_e4m3fn
BF16 = mybir.dt.bfloat16
NP_BF16 = ml_dtypes.bfloat16

_COMPILED = {}


def _build(repeat: int = 1):
    f32 = mybir.dt.float32
    nc = bacc.Bacc(
        "TRN2",
        target_bir_lowering=False,
        debug=False,
        enable_asserts=False,
        num_devices=NCORES,
        num_swdge_queues=4,
    )
    h0T = nc.dram_tensor("h0T", [128, NB, D], BF16, kind="ExternalInput")
    amat = nc.dram_tensor("amat", [128, NB, NSP], A_DT, kind="ExternalInput")
    wmat = nc.dram_tensor("wmat", [128, 2, D], BF16, kind="ExternalInput")
    brep = nc.dram_tensor("brep", [128, 2, D], f32, kind="ExternalInput")
    out1 = nc.dram_tensor("out1", [NSP, D], f32, kind="ExternalOutput")
    out2 = nc.dram_tensor("out2", [NSP, D], f32, kind="ExternalOutput")

    with tile.TileContext(nc) as tc:
        with (
            tc.tile_pool(name="const", bufs=1) as const,
            tc.tile_pool(name="src", bufs=1) as srcp,
            tc.tile_pool(name="a", bufs=3) as ap,
            tc.tile_pool(name="agg", bufs=2) as aggp,
            tc.tile_pool(name="pg", bufs=1, space="PSUM") as pg,
            tc.tile_pool(name="ph", bufs=2, space="PSUM") as ph,
            tc.tile_pool(name="h", bufs=1) as hp,
            tc.tile_pool(name="dram", bufs=repeat, space="DRAM") as dram,
        ):
            w_sb = const.tile([128, 2, D], BF16)
            nc.sync.dma_start(w_sb[:], wmat[:])
            b_sb = const.tile([128, 2, D], f32)
            nc.sync.dma_start(b_sb[:], brep[:])

            h0sb = srcp.tile([128, NB, D], BF16)
            h1all = srcp.tile([128, NCORES, NCH, D], BF16)
            h1f = hp.tile([128, NCH, D], f32)
            h1b = hp.tile([128, NCH, D], BF16)
            h2f = hp.tile([128, NCH, D], f32)

            def layer(src_of, lidx, hf, hb):
                psg = pg.tile([128, NSP], f32, tag="psg")
                for jg in range(NGRP):
                    a_t = ap.tile([128, GRP, NSP], A_DT, tag="a")
                    eng = nc.sync if jg % 2 == 0 else nc.scalar
                    eng.dma_start(
                        a_t[:], amat[:, GRP * jg : GRP * jg + GRP, :]
                    )
                    for gi in range(GRP):
                        g = GRP * jg + gi
                        for s in range(NSP // 512):
                            nc.tensor.matmul(
                                psg[:, 512 * s : 512 * s + 512],
                                lhsT=src_of(g),
                                rhs=a_t[:, gi, 512 * s : 512 * s + 512],
                                start=(g == 0),
                                stop=(g == NB - 1),
                            )
                aggT = aggp.tile([128, NSP], BF16, tag="aggT")
                nc.vector.tensor_copy(aggT[:], psg[:])
                for nt in range(NCH):
                    pht = ph.tile([128, D], f32, tag="pht")
                    nc.tensor.matmul(
                        pht[:],
                        lhsT=aggT[:, 128 * nt : 128 * nt + 128],
                        rhs=w_sb[:, lidx, :],
                        start=True,
                        stop=True,
                    )
                    nc.vector.tensor_add(hf[:, nt, :], pht[:], b_sb[:, lidx, :])
                    nc.vector.tensor_scalar_max(hf[:, nt, :], hf[:, nt, :], 0.0)
                    if hb is not None:
                        nc.scalar.copy(hb[:, nt, :], hf[:, nt, :])

            for _ in range(repeat):
                nc.scalar.dma_start(h0sb[:], h0T[:])
                layer(lambda g: h0sb[:, g, :], 0, h1f, h1b)
                ag_in = dram.tile([128, NCH, D], BF16, tag="ag_in")
                ag_out = dram.tile(
                    [NCORES, 128, NCH, D], BF16, addr_space="Shared", tag="ag_out"
                )
                nc.sync.dma_start(ag_in[:], h1b[:])
                nc.gpsimd.collective_compute(
                    "AllGather",
                    mybir.AluOpType.bypass,
                    replica_groups=[list(range(NCORES))],
                    ins=[ag_in.opt()],
                    outs=[ag_out.opt()],
                )
                nc.scalar.dma_start(
                    h1all[:], ag_out[:].rearrange("c p b d -> p c b d")
                )
                layer(lambda g: h1all[:, g // NCH, g % NCH, :], 1, h2f, None)
            nc.sync.dma_start(out1[:].rearrange("(b p) d -> p b d", p=128), h1f[:])
            nc.sync.dma_start(out2[:].rearrange("(b p) d -> p b d", p=128), h2f[:])
    nc.compile()
    return nc


def _get_compiled(repeat: int = 1):
    if repeat not in _COMPILED:
        _COMPILED[repeat] = _build(repeat)
    return _COMPILED[repeat]


def _prep_inputs(adjacency, graph, W, b):
    adj = np.asarray(adjacency).astype(np.int64)  # [K, N]
    graph = np.asarray(graph, dtype=np.float32)  # [1, N, D]
    W = np.asarray(W, dtype=np.float32)  # [3, D, D]
    b = np.asarray(b, dtype=np.float32)  # [3, D]

    # h0 in padded-global (p, block, d) layout, pad rows zero
    h0pad = np.zeros((NCORES, NSP, D), np.float32)
    h0pad[:, :NS] = graph[0].reshape(NCORES, NS, D)
    # row j' = c*2560 + bl*128 + p  ->  h0T[p, c*20+bl, :]
    h0T = np.ascontiguousarray(
        h0pad.reshape(NCORES, NCH, 128, D).transpose(2, 0, 1, 3).reshape(128, NB, D)
    ).astype(NP_BF16)

    w_host = np.ascontiguousarray(
        np.stack([W[1] / K, W[2] / K]).transpose(1, 0, 2)
    ).astype(NP_BF16)  # [128 d_in, 2, 128 d_out]
    b_host = np.ascontiguousarray(
        np.broadcast_to(b[1:3][:, None, :], (2, 128, D)).transpose(1, 0, 2)
    ).astype(np.float32)  # [128, 2, 128]

    jj = np.minimum(np.arange(NSP), NS - 1)
    in_maps = []
    for c in range(NCORES):
        ga = adj[:, NS * c + jj]  # [K, NSP] global neighbor ids
        jp = (ga // NS) * NSP + (ga % NS)  # padded-global coords
        cnt = np.zeros(NB * 128 * NSP, np.uint8)
        np.add.at(cnt, (jp.ravel() * NSP + np.tile(np.arange(NSP), K)), 1)
        amat = np.ascontiguousarray(
            cnt.reshape(NB, 128, NSP).transpose(1, 0, 2)
        ).astype(A_NP_DT)
        in_maps.append(
            {"h0T": h0T, "amat": amat, "wmat": w_host, "brep": b_host}
        )
    return in_maps


def kernel(adjacency, graph, W, b):
    graph = np.asarray(graph, dtype=np.float32)
    in_maps = _prep_inputs(adjacency, graph, W, b)
    nc = _get_compiled(repeat=1)
    res = run_bass_kernel_spmd(nc, in_maps, core_ids=list(range(NCORES)), trace=False)
    h1 = np.concatenate([res.results[c]["out1"][:NS] for c in range(NCORES)], axis=0)
    h2 = np.concatenate([res.results[c]["out2"][:NS] for c in range(NCORES)], axis=0)
    out = np.stack([graph[0], h1, h2], axis=0)[None]  # [1, 3, N, D]
    return out.astype(np.float32)


# revision 3
# speedup vs baseline: 1.4557x; 1.4557x over previous
"""GNN message passing via dense aggregation-matrix matmul (8 cores, SPMD).

Reference: h_t = relu(mean_k h_{t-1}[adj[k,n]] @ W[t] + b[t]), out = [h0,h1,h2].

The neighbor gather+mean is multiplication by a sparse count matrix C
([N_out, N_in], 32 nonzeros per row, values = duplicate counts):
    agg = (C @ H) / 32;  h = relu(agg @ (W/32') ...) with 1/32 folded into W.
SWDGE per-row gathers run at ~1.4us/row on this HW (236ms total), so instead
we materialize C^T densely per core (host-side, from the static adjacency)
and stream it through the PE at full DMA bandwidth:

Per core (2500 output nodes, padded to 2560 = 20 blocks of 128):
    psum[d, n] += H_block[p=j, d].T @ CT_block[p=j, n]   (160 j-blocks, fp8 CT)
    h[n, e] = relu(aggT[d, n].T @ (W/32)[d, e] + b)      (20 n-tiles)
Node coords use the AllGather-padded global space j' = core*2560 + local
for BOTH layers, so one CT layout serves layer 1 (src = padded graph) and
layer 2 (src = AllGathered h1).
"""

import numpy as np
import ml_dtypes

import concourse.bacc as bacc
import concourse.mybir as mybir
import concourse.tile as tile
from concourse.bass_utils import run_bass_kernel_spmd

N, K, D = 20000, 32, 128
NCORES = 8
NS = N // NCORES  # 2500 real nodes per core
CHUNK = 128
NCH = (NS + CHUNK - 1) // CHUNK  # 20 blocks per core
NSP = NCH * CHUNK  # 2560 padded nodes per core
NB = NCORES * NCH  # 160 global padded j-blocks
GRP = 4  # A^T blocks per DMA
NGRP = NB // GRP  # 40 DMA groups per layer

A_DT = mybir.dt.float8e4
A_NP_DT = ml_dtypes.float8_e4m3fn
DR = mybir.MatmulPerfMode.DoubleRow
BF16 = mybir.dt.bfloat16
NP_BF16 = ml_dtypes.bfloat16

_COMPILED = {}


def _build(repeat: int = 1):
    f32 = mybir.dt.float32
    nc = bacc.Bacc(
        "TRN2",
        target_bir_lowering=False,
        debug=False,
        enable_asserts=False,
        num_devices=NCORES,
        num_swdge_queues=4,
    )
    h0T = nc.dram_tensor("h0T", [128, NB, D], A_DT, kind="ExternalInput")
    amat = nc.dram_tensor("amat", [128, NB, NSP], A_DT, kind="ExternalInput")
    wmat = nc.dram_tensor("wmat", [128, 2, D], BF16, kind="ExternalInput")
    brep = nc.dram_tensor("brep", [128, 2, D], f32, kind="ExternalInput")
    out1 = nc.dram_tensor("out1", [NSP, D], f32, kind="ExternalOutput")
    out2 = nc.dram_tensor("out2", [NSP, D], f32, kind="ExternalOutput")

    with tile.TileContext(nc) as tc:
        with (
            tc.tile_pool(name="const", bufs=1) as const,
            tc.tile_pool(name="src", bufs=1) as srcp,
            tc.tile_pool(name="a", bufs=6) as ap,
            tc.tile_pool(name="agg", bufs=2) as aggp,
            tc.tile_pool(name="pg", bufs=1, space="PSUM") as pg,
            tc.tile_pool(name="ph", bufs=2, space="PSUM") as ph,
            tc.tile_pool(name="h", bufs=1) as hp,
            tc.tile_pool(name="dram", bufs=repeat, space="DRAM") as dram,
        ):
            w_sb = const.tile([128, 2, D], BF16)
            nc.sync.dma_start(w_sb[:], wmat[:])
            b_sb = const.tile([128, 2, D], f32)
            nc.sync.dma_start(b_sb[:], brep[:])

            h0sb = [
                srcp.tile([128, NB // 4, D], A_DT, name=f"h0sb{i}")
                for i in range(4)
            ]
            h1all = [
                srcp.tile([128, NCORES // 2, NCH, D], A_DT, name=f"h1all{i}")
                for i in range(2)
            ]
            h1f = hp.tile([128, NCH, D], f32)
            h1b = hp.tile([128, NCH, D], A_DT)
            h2f = hp.tile([128, NCH, D], f32)

            def layer(src_of, lidx, hf, hb):
                psg = pg.tile([128, NSP], f32, tag="psg")
                for jg in range(NGRP):
                    a_t = ap.tile([128, GRP, NSP], A_DT, tag="a")
                    eng = nc.sync if jg % 2 == 0 else nc.scalar
                    eng.dma_start(
                        a_t[:], amat[:, GRP * jg : GRP * jg + GRP, :]
                    )
                    for gi2 in range(GRP // 2):
                        gg = (GRP // 2) * jg + gi2
                        for s in range(NSP // 512):
                            nc.tensor.matmul(
                                psg[:, 512 * s : 512 * s + 512],
                                lhsT=src_of(gg),
                                rhs=a_t[
                                    :, 2 * gi2 : 2 * gi2 + 2, 512 * s : 512 * s + 512
                                ],
                                start=(gg == 0),
                                stop=(gg == NB // 2 - 1),
                                perf_mode=DR,
                            )
                aggT = aggp.tile([128, NSP], BF16, tag="aggT")
                nc.vector.tensor_copy(aggT[:], psg[:])
                for nt in range(NCH):
                    pht = ph.tile([128, D], f32, tag="pht")
                    nc.tensor.matmul(
                        pht[:],
                        lhsT=aggT[:, 128 * nt : 128 * nt + 128],
                        rhs=w_sb[:, lidx, :],
                        start=True,
                        stop=True,
                    )
                    nc.vector.tensor_add(hf[:, nt, :], pht[:], b_sb[:, lidx, :])
                    nc.vector.tensor_scalar_max(hf[:, nt, :], hf[:, nt, :], 0.0)
                    if hb is not None:
                        nc.scalar.copy(hb[:, nt, :], hf[:, nt, :])

            Q = NB // 4
            for _ in range(repeat):
                for q in range(4):
                    eng = nc.scalar if q % 2 == 0 else nc.sync
                    eng.dma_start(h0sb[q][:], h0T[:, Q * q : Q * q + Q, :])
                layer(
                    lambda gg: h0sb[2 * gg // Q][
                        :, (2 * gg) % Q : (2 * gg) % Q + 2, :
                    ],
                    0,
                    h1f,
                    h1b,
                )
                ag_in = dram.tile([128, NCH, D], A_DT, tag="ag_in")
                ag_out = dram.tile(
                    [NCORES, 128, NCH, D], A_DT, addr_space="Shared", tag="ag_out"
                )
                nc.sync.dma_start(ag_in[:], h1b[:])
                nc.gpsimd.collective_compute(
                    "AllGather",
                    mybir.AluOpType.bypass,
                    replica_groups=[list(range(NCORES))],
                    ins=[ag_in.opt()],
                    outs=[ag_out.opt()],
                )
                half = NCORES // 2
                for q in range(2):
                    eng = nc.scalar if q == 0 else nc.sync
                    eng.dma_start(
                        h1all[q][:],
                        ag_out[half * q : half * q + half].rearrange(
                            "c p b d -> p c b d"
                        ),
                    )
                layer(
                    lambda gg: h1all[2 * gg // (half * NCH)][
                        :, (2 * gg // NCH) % half, (2 * gg) % NCH : (2 * gg) % NCH + 2, :
                    ],
                    1,
                    h2f,
                    None,
                )
            nc.sync.dma_start(out1[:].rearrange("(b p) d -> p b d", p=128), h1f[:])
            nc.sync.dma_start(out2[:].rearrange("(b p) d -> p b d", p=128), h2f[:])
    nc.compile()
    return nc


def _get_compiled(repeat: int = 1):
    if repeat not in _COMPILED:
        _COMPILED[repeat] = _build(repeat)
    return _COMPILED[repeat]


def _prep_inputs(adjacency, graph, W, b):
    adj = np.asarray(adjacency).astype(np.int64)  # [K, N]
    graph = np.asarray(graph, dtype=np.float32)  # [1, N, D]
    W = np.asarray(W, dtype=np.float32)  # [3, D, D]
    b = np.asarray(b, dtype=np.float32)  # [3, D]

    # h0 in padded-global (p, block, d) layout, pad rows zero
    h0pad = np.zeros((NCORES, NSP, D), np.float32)
    h0pad[:, :NS] = graph[0].reshape(NCORES, NS, D)
    # row j' = c*2560 + bl*128 + p  ->  h0T[p, c*20+bl, :]
    h0T = np.ascontiguousarray(
        h0pad.reshape(NCORES, NCH, 128, D).transpose(2, 0, 1, 3).reshape(128, NB, D)
    ).astype(A_NP_DT)

    w_host = np.ascontiguousarray(
        np.stack([W[1] / K, W[2] / K]).transpose(1, 0, 2)
    ).astype(NP_BF16)  # [128 d_in, 2, 128 d_out]
    b_host = np.ascontiguousarray(
        np.broadcast_to(b[1:3][:, None, :], (2, 128, D)).transpose(1, 0, 2)
    ).astype(np.float32)  # [128, 2, 128]

    jj = np.minimum(np.arange(NSP), NS - 1)
    in_maps = []
    for c in range(NCORES):
        ga = adj[:, NS * c + jj]  # [K, NSP] global neighbor ids
        jp = (ga // NS) * NSP + (ga % NS)  # padded-global coords
        cnt = np.zeros(NB * 128 * NSP, np.uint8)
        np.add.at(cnt, (jp.ravel() * NSP + np.tile(np.arange(NSP), K)), 1)
        amat = np.ascontiguousarray(
            cnt.reshape(NB, 128, NSP).transpose(1, 0, 2)
        ).astype(A_NP_DT)
        in_maps.append(
            {"h0T": h0T, "amat": amat, "wmat": w_host, "brep": b_host}
        )
    return in_maps


def kernel(adjacency, graph, W, b):
    graph = np.asarray(graph, dtype=np.float32)
    in_maps = _prep_inputs(adjacency, graph, W, b)
    nc = _get_compiled(repeat=1)
    res = run_bass_kernel_spmd(nc, in_maps, core_ids=list(range(NCORES)), trace=False)
    h1 = np.concatenate([res.results[c]["out1"][:NS] for c in range(NCORES)], axis=0)
    h2 = np.concatenate([res.results[c]["out2"][:NS] for c in range(NCORES)], axis=0)
    out = np.stack([graph[0], h1, h2], axis=0)[None]  # [1, 3, N, D]
    return out.astype(np.float32)
